# revision 1
# baseline (speedup 1.0000x reference)
"""Causal single-head attention, data-parallel across 8 TRN2 NeuronCores.

Problem: x [512, 128, 512] f32, Wq/Wk/Wv [64, 512] f32.
  Q = x @ Wq.T; K = x @ Wk.T; V = x @ Wv.T     (per batch, [T=128, H=64])
  out = softmax(causal(Q K^T / 8)) @ V          ([T, H])

Sharding: batch dim (512) split across 8 cores, 64 batches/core, no
collectives.  Host prep (layout only): x is cast to bf16 and laid out
as [tile, c-part, c-chunk, token] so each 512-token tile (4 batches)
feeds N=512 matmuls directly; weights pre-transposed to [c, 3H] bf16
(unscaled -- the 1/8 softmax scale is folded into the exp activation).

Per-core kernel (16 token-tiles of 4 batches, bf16 compute, f32 out):
  - QK projection: one [128,512]-out matmul per C-chunk (M=128: Q rows
    0-63, K rows 64-127), N=512 amortizes the per-matmul overhead.
  - V projection: col-tiled pairs (tile_position=(0,64g)) so two N=256
    matmuls run concurrently on half-arrays.
  - PE matmul operands must share one base partition (walrus: "Fmap
    and Weight must start at the same partition"), but DVE/ACT copies
    CAN cross partition offsets (verified on HW), so K is evacuated
    from qk_ps partitions 64-127 straight to a base-0 tile by ACT.
  - scores are computed TRANSPOSED: S^T[s,t] = sum_h KT[h,s] QT[h,t]
    via lhsT=K^T, rhs=Q^T (both h-partitioned from the projection), so
    exp(S^T) IS P^T and feeds the PV matmul with no P transpose.
  - V natural layout via one PE transpose per stacked batch-pair.
  - ACT exp applies the 1/8 scale; GPSIMD affine_select applies the
    causal mask (keep t >= s in [s, b, t] layout).
  - V gets a ones column appended so the PV matmul also produces the
    softmax row-sums; DVE reciprocal + broadcast multiply normalizes.
  - PSUM->SBUF evacuation is split DVE/ACT to balance engine load;
    x tile loads go through the sync-engine HWDGE ring (SWDGE
    descriptor generation was blocking GPSIMD ~1us per tile).
  - 3-stage software pipeline (proj i | mid i-1 | back i-2) keeps the
    PE queue from head-blocking on ACT/DVE work.
"""

import contextlib

import numpy as np
import ml_dtypes

import concourse.mybir as mybir
import concourse.tile as tile
from concourse import bacc
from concourse.bass_utils import run_bass_kernel_spmd
from concourse.masks import make_identity

B, T, C, H = 512, 128, 512, 64
NCORES = 8
BPC = B // NCORES          # 64 batches per core
NBT = 4                    # batches per token tile
NT = BPC // NBT            # 16 token tiles
NTOK = NBT * T             # 512 tokens per tile
KCH = C // 128             # 4 contraction chunks

BF16 = mybir.dt.bfloat16
F32 = mybir.dt.float32

_cache = {}


def _build(reps=1):
    nc = bacc.Bacc(
        "TRN2", target_bir_lowering=False, debug=False, enable_asserts=False
    )
    x_d = nc.dram_tensor(
        "x", [NT, 128, KCH, NTOK], BF16, kind="ExternalInput"
    ).ap()
    w_d = nc.dram_tensor("w", [128, KCH, 3 * H], BF16, kind="ExternalInput").ap()
    out_d = nc.dram_tensor("out", [NT, T, NBT, H], F32, kind="ExternalOutput").ap()

    with tile.TileContext(nc) as tc:
        with (
            tc.tile_pool(name="const", bufs=1) as cpool,
            tc.tile_pool(name="xt", bufs=3) as xtpool,
            tc.tile_pool(name="qsb", bufs=2) as qpool,
            tc.tile_pool(name="ksb", bufs=2) as kpool,
            tc.tile_pool(name="vtsb", bufs=2) as vtpool,
            tc.tile_pool(name="vsb", bufs=2) as vpool,
            tc.tile_pool(name="psb", bufs=2) as ppool,
            tc.tile_pool(name="rsb", bufs=2) as rpool,
            tc.tile_pool(name="osb", bufs=2) as opool,
            tc.tile_pool(name="qkps", bufs=2, space="PSUM") as qkps,
            tc.tile_pool(name="vps", bufs=2, space="PSUM") as vps,
            tc.tile_pool(name="vtps", bufs=1, space="PSUM") as vtps,
            tc.tile_pool(name="sps", bufs=2, space="PSUM") as sps,
            tc.tile_pool(name="ops", bufs=1, space="PSUM") as ops,
        ):
            w_sb = cpool.tile([128, KCH, 3 * H], BF16)
            nc.sync.dma_start(out=w_sb, in_=w_d)
            ident = cpool.tile([128, 128], BF16)
            make_identity(nc, ident)
            st = {}

            def dma_in(i, split=False):
                xt = xtpool.tile([128, KCH, NTOK], BF16)
                if split:
                    # per-chunk DMAs so the first proj matmul only waits
                    # for 1/4 of the tile (cold-start latency cut)
                    for j in range(KCH):
                        nc.scalar.dma_start(out=xt[:, j, :], in_=x_d[i][:, j, :])
                else:
                    nc.scalar.dma_start(out=xt, in_=x_d[i])
                st[i] = {"xt": xt}

            def stage_proj(i):
                xt = st[i]["xt"]
                qk_ps = qkps.tile([128, NTOK], F32)
                for j in range(KCH):
                    nc.tensor.matmul(
                        qk_ps,
                        w_sb[:, j, 0:128],
                        xt[:, j, :],
                        start=(j == 0),
                        stop=(j == KCH - 1),
                    )
                v_ps = vps.tile([128, NTOK // 2], F32)
                for j in range(KCH):
                    for g in range(2):
                        nc.tensor.matmul(
                            v_ps[64 * g : 64 * g + 64, :],
                            w_sb[:, j, 128:192],
                            xt[:, j, 256 * g : 256 * g + 256],
                            start=(j == 0),
                            stop=(j == KCH - 1),
                            tile_position=(0, 64 * g),
                            skip_group_check=True,
                        )
                st[i]["qk_ps"] = qk_ps
                st[i]["v_ps"] = v_ps

            def stage_mid(i):
                s = st[i]
                # evacuate V^T first (PE transposes are next on the PE
                # queue), then Q (DVE, aligned) and K (ACT, cross-partition)
                vt_sb = vtpool.tile([128, NTOK // 2], BF16)
                nc.vector.tensor_copy(vt_sb, s["v_ps"])
                q_sb = qpool.tile([64, NTOK], BF16)
                nc.vector.tensor_copy(q_sb, s["qk_ps"][0:64, :])
                k_sb = kpool.tile([64, NTOK], BF16)
                nc.scalar.copy(out=k_sb, in_=s["qk_ps"][64:128, :])
                # V natural layout: transpose stacked pairs [VT_b; VT_b+2]
                vt_ps = vtps.tile([T, 2, 128], BF16)
                for p in range(2):
                    nc.tensor.transpose(
                        vt_ps[:, p, :], vt_sb[:, 128 * p : 128 * p + 128], ident
                    )
                v_sb = vpool.tile([T, NBT, H + 1], BF16)
                nc.scalar.copy(
                    out=v_sb.rearrange("t (g p) h -> t p g h", g=2)[:, :, :, 0:H],
                    in_=vt_ps.rearrange("t p (g h) -> t p g h", g=2),
                )
                nc.gpsimd.memset(v_sb[:, :, H : H + 1], 1.0)
                s["v"] = v_sb
                # transposed scores: S^T[s,t] per batch, K=64 contraction
                s_ps = sps.tile([T, NBT, T], F32)
                for b in range(NBT):
                    nc.tensor.matmul(
                        s_ps[:, b, :],
                        k_sb[:, T * b : T * b + T],
                        q_sb[:, T * b : T * b + T],
                        start=True,
                        stop=True,
                    )
                p_sb = ppool.tile([T, NBT, T], BF16)
                nc.scalar.activation(
                    out=p_sb,
                    in_=s_ps,
                    func=mybir.ActivationFunctionType.Exp,
                    scale=0.125,
                )
                # causal: keep where t - s >= 0 (layout [s, b, t])
                nc.gpsimd.affine_select(
                    out=p_sb,
                    in_=p_sb,
                    pattern=[[0, NBT], [1, T]],
                    compare_op=mybir.AluOpType.is_ge,
                    fill=0.0,
                    base=0,
                    channel_multiplier=-1,
                )
                s["p"] = p_sb

            def stage_back(i):
                s = st[i]
                o_ps = ops.tile([T, NBT, H + 1], F32)
                for b in range(NBT):
                    nc.tensor.matmul(
                        o_ps[:, b, :],
                        s["p"][:, b, :],
                        s["v"][:, b, :],
                        start=True,
                        stop=True,
                    )
                r_sb = rpool.tile([T, NBT, 1], F32)
                nc.vector.reciprocal(out=r_sb, in_=o_ps[:, :, H : H + 1])
                o_sb = opool.tile([T, NBT, H], F32)
                nc.vector.tensor_mul(
                    o_sb, o_ps[:, :, 0:H], r_sb.to_broadcast([T, NBT, H])
                )
                nc.sync.dma_start(out=out_d[i], in_=o_sb)
                del st[i]

            loop = (
                tc.For_i(0, reps, 1, hint_engines=tuple(nc.engines))
                if reps > 1
                else contextlib.nullcontext()
            )
            with loop:
                for i in range(NT + 2):
                    if i < NT:
                        if i == 0:
                            dma_in(0, split=True)
                            dma_in(1, split=True)
                            dma_in(2)
                        elif i + 2 < NT:
                            dma_in(i + 2)
                        stage_proj(i)
                    if 1 <= i <= NT:
                        stage_mid(i - 1)
                    if i >= 2:
                        stage_back(i - 2)

    nc.compile()
    return nc


def _prep_inputs(x, Wq, Wk, Wv):
    w = np.concatenate(
        [np.asarray(Wq).T, np.asarray(Wk).T, np.asarray(Wv).T], axis=1
    )  # [C, 3H]
    w = np.ascontiguousarray(
        w.reshape(KCH, 128, 3 * H).transpose(1, 0, 2)
    ).astype(ml_dtypes.bfloat16)  # [128, KCH, 3H]
    # x [B, T, C] -> per-core [NT, 128(c-part), KCH, NTOK], token = b*T + t
    xt = np.asarray(x, dtype=np.float32).reshape(NCORES, NT, NBT, T, KCH, 128)
    xt = np.ascontiguousarray(xt.transpose(0, 1, 5, 4, 2, 3)).astype(
        ml_dtypes.bfloat16
    )
    xt = xt.reshape(NCORES, NT, 128, KCH, NTOK)
    return [{"x": xt[i], "w": w} for i in range(NCORES)]


def _run(in_maps, **kw):
    if "nc" not in _cache:
        _cache["nc"] = _build()
    return run_bass_kernel_spmd(
        _cache["nc"], in_maps, core_ids=list(range(NCORES)), **kw
    )


def kernel(x, Wq, Wk, Wv):
    res = _run(_prep_inputs(x, Wq, Wk, Wv))
    outs = []
    for r in res.results:
        o = r["out"].astype(np.float32)  # [NT, T, NBT, H]
        outs.append(o.transpose(0, 2, 1, 3).reshape(BPC, T, H))
    return np.ascontiguousarray(np.concatenate(outs, axis=0))

